# revision 18
# baseline (speedup 1.0000x reference)
"""Distributed APPNP (GCN propagation) kernel for 8 TRN2 NeuronCores.

Algorithm (reference): h = relu(x@W+b); 50 x { h <- 0.9 * A_hat h + 0.1 * x0 }
with A_hat = D^-1/2 (A+I) D^-1/2.

Reformulated with g = dinv * h so the per-edge work is a pure row gather:
  raw[i] = sum_{j -> i} g[j]      (unweighted, incl. self loop)
  g'     = (0.9 dinv^2) * raw + (0.1 dinv) * x0      (intermediate steps)
  h_out  = (0.9 dinv)   * raw + 0.1 * x0             (final step)

Distribution: nodes dst-sharded 8 x 6250. Each core keeps a full replicated
g-table in DRAM (refreshed by AllGather each step) and computes raw for its
shard with `dma_gather` over padded "waves": nodes are sorted by in-degree
descending so wave k (the k-th incoming edge of every node) is a prefix of
the accumulator; each gathered wave is accumulated with one DVE add.

The int16 gather-index limit (< 32768) forces a lo/hi table split: edges
from shards 0-4 (lo) and shards 5-7 (hi) use separate accumulators with
separate degree-sorted orders; acc_H is merged into acc_L order once per
step via a small local permutation gather.
"""

import sys

sys.path.insert(0, "/opt/trn_rl_repo")

import numpy as np

N, E, CIN, COUT = 50000, 800000, 256, 64
NC = 8
SH = N // NC            # 6250 real nodes per shard
SHP = 6272              # padded shard rows (49*128)
TILES = SHP // 128      # 49
LO_N = 5 * SH           # node ids < LO_N live in the lo table
TBL = NC * SHP + 2      # [Z][8*6272 rows][Z2]
LO_ROWS = 1 + 5 * SHP   # lo table = rows [0, 31361)
HI_BASE = 1 + 5 * SHP   # first hi node row
HI_ROWS = TBL - HI_BASE  # 18817 rows (incl Z2 at the end)
HI_ZIDX = HI_ROWS - 1   # 18816
K = 50
CH = 12288              # gather chunk slots (capped by SWDGE desc ring)


def _build_host(x, edge_index, W, b):
    """All index preprocessing. Returns (in_maps, schedule)."""
    x = np.ascontiguousarray(np.asarray(x, dtype=np.float32))
    W = np.ascontiguousarray(np.asarray(W, dtype=np.float32))
    b = np.ascontiguousarray(np.asarray(b, dtype=np.float32))
    src = np.asarray(edge_index[0]).astype(np.int64)
    dst = np.asarray(edge_index[1]).astype(np.int64)

    deg = np.bincount(dst, minlength=N).astype(np.float64) + 1.0
    dinv = (1.0 / np.sqrt(deg)).astype(np.float32)

    cores = []
    for c in range(NC):
        m = (dst >= c * SH) & (dst < (c + 1) * SH)
        es = np.concatenate([src[m], np.arange(c * SH, (c + 1) * SH)])
        ed = np.concatenate([dst[m] - c * SH, np.arange(SH)])
        is_lo = es < LO_N
        deg_lo = np.bincount(ed[is_lo], minlength=SH)
        deg_hi = np.bincount(ed[~is_lo], minlength=SH)
        rankL = np.argsort(-deg_lo, kind="stable")
        rankH = np.argsort(-deg_hi, kind="stable")
        posL = np.empty(SH, np.int64); posL[rankL] = np.arange(SH)
        posH = np.empty(SH, np.int64); posH[rankH] = np.arange(SH)
        cores.append(dict(es=es, ed=ed, is_lo=is_lo, deg_lo=deg_lo,
                          deg_hi=deg_hi, rankL=rankL, rankH=rankH,
                          posL=posL, posH=posH))

    table_row = np.empty(N, np.int64)
    for c in range(NC):
        table_row[c * SH:(c + 1) * SH] = 1 + c * SHP + cores[c]["posL"]

    # common (max over cores) padded wave sizes
    KL = max(int(c["deg_lo"].max()) for c in cores)
    KH = max(int(c["deg_hi"].max()) for c in cores)
    NL = [min(-(-max(int((c["deg_lo"] > k).sum()) for c in cores) // 128) * 128,
              SHP) for k in range(KL)]
    NH = [min(-(-max(int((c["deg_hi"] > k).sum()) for c in cores) // 128) * 128,
              SHP) for k in range(KH)]

    def wrap_idx(a):
        # idx j -> partition j%16, free j//16; replicated to 128 partitions
        w = a.reshape(-1, 16).T
        return np.tile(w, (8, 1)).astype(np.int16)

    in_maps = []
    for c in range(NC):
        d = cores[c]
        for (sel, pos, NWS, key, padv) in [
            (d["is_lo"], d["posL"], NL, "WL", 0),
            (~d["is_lo"], d["posH"], NH, "WH", HI_ZIDX),
        ]:
            e_s = d["es"][sel]
            p = pos[d["ed"][sel]]
            o = np.argsort(p, kind="stable")
            p_s = p[o]; s_s = e_s[o]
            first = np.searchsorted(p_s, p_s, side="left")
            slot = np.arange(len(p_s)) - first
            Wm = np.full((len(NWS), SHP), padv, np.int32)
            tr = table_row[s_s]
            Wm[slot, p_s] = np.where(s_s < LO_N, tr, tr - HI_BASE)
            d[key] = np.concatenate([Wm[k, :NWS[k]] for k in range(len(NWS))])
        pm = np.zeros(SHP, np.int32)
        pm[:SH] = d["posH"][d["rankL"]]
        dv = np.zeros(SHP, np.float32)
        dv[:SH] = dinv[c * SH + d["rankL"]]

        xp = np.zeros((SHP, CIN), np.float32)
        xp[:SH] = x[c * SH + d["rankL"]]
        xt = np.concatenate([xp.T, np.ones((1, SHP), np.float32)], axis=0)
        waug = np.concatenate([W, b[None, :]], axis=0)

        ex = lambda v: np.repeat(v[:, None], COUT, axis=1).astype(np.float32)
        in_maps.append({
            "xt": np.ascontiguousarray(xt),
            "waug": np.ascontiguousarray(waug),
            "idxl": wrap_idx(d["WL"]),
            "idxh": wrap_idx(d["WH"]),
            "idxp": wrap_idx(pm),
            "da": ex(0.9 * dv * dv),
            "db": ex(0.9 * dv),
            "di": ex(dv),
        })
    perms = [c["rankL"] for c in cores]
    return in_maps, (tuple(NL), tuple(NH)), perms


def _chunks(NWS):
    """Cut concatenated waves into gather chunks; return list of
    (start, length, [(buf_tile0, buf_tile1, acc_tile0, acc_tile1), ...])."""
    offs = np.cumsum([0] + list(NWS))
    total = int(offs[-1])
    out = []
    a = 0
    while a < total:
        b = min(a + CH, total)
        segs = []
        for k in range(len(NWS)):
            s0 = max(a, int(offs[k])); s1 = min(b, int(offs[k + 1]))
            if s1 > s0:
                segs.append(((s0 - a) // 128, (s1 - a) // 128,
                             (s0 - int(offs[k])) // 128,
                             (s1 - int(offs[k])) // 128))
        out.append((a, b - a, segs))
        a = b
    return out


def _build_graph(NL, NH, steps=K):
    import concourse.bacc as bacc
    import concourse.mybir as mybir
    import concourse.tile as tile

    f32 = mybir.dt.float32
    i16 = mybir.dt.int16

    chunksL = _chunks(NL)
    chunksH = _chunks(NH)
    FL = sum(NL) // 16
    FH = sum(NH) // 16

    nc = bacc.Bacc("TRN2", target_bir_lowering=False, debug=False,
                   enable_asserts=False, num_devices=NC)

    xt_d = nc.dram_tensor("xt", [CIN + 1, SHP], f32, kind="ExternalInput")
    waug_d = nc.dram_tensor("waug", [CIN + 1, COUT], f32, kind="ExternalInput")
    idxl_d = nc.dram_tensor("idxl", [128, FL], i16, kind="ExternalInput")
    idxh_d = nc.dram_tensor("idxh", [128, FH], i16, kind="ExternalInput")
    idxp_d = nc.dram_tensor("idxp", [128, SHP // 16], i16, kind="ExternalInput")
    da_d = nc.dram_tensor("da", [SHP, COUT], f32, kind="ExternalInput")
    db_d = nc.dram_tensor("db", [SHP, COUT], f32, kind="ExternalInput")
    di_d = nc.dram_tensor("di", [SHP, COUT], f32, kind="ExternalInput")
    out_d = nc.dram_tensor("out", [SHP, COUT], f32, kind="ExternalOutput")

    import concourse.bass as bass

    def to_pf(dram):  # [SHP, 64] dram -> [128, 49, 64] partition-major view
        ap = dram if isinstance(dram, bass.AP) else dram.ap()
        return ap.rearrange("(t p) f -> p t f", p=128)

    with tile.TileContext(nc) as tc:
        with (
            tc.tile_pool(name="dram", bufs=1, space="DRAM") as dpool,
            tc.tile_pool(name="res", bufs=1) as res,
            tc.tile_pool(name="gb", bufs=3) as gbp,
            tc.tile_pool(name="ps", bufs=2, space="PSUM") as psp,
        ):
            table = dpool.tile([TBL, COUT], f32)
            ag_in = dpool.tile([SHP, COUT], f32)
            hscr = dpool.tile([SHP, COUT], f32)

            # resident SBUF
            idxl = res.tile([128, FL], i16)
            idxh = res.tile([128, FH], i16)
            idxp = res.tile([128, SHP // 16], i16)
            da = res.tile([128, TILES, COUT], f32)
            x0q = res.tile([128, TILES, COUT], f32)
            cexp = res.tile([128, TILES, COUT], f32)
            accL = res.tile([128, TILES, COUT], f32)
            accH = res.tile([128, TILES, COUT], f32)
            zrow = res.tile([2, COUT], f32)

            nc.sync.dma_start(idxl[:, :], idxl_d[:, :])
            nc.sync.dma_start(idxh[:, :], idxh_d[:, :])
            nc.sync.dma_start(idxp[:, :], idxp_d[:, :])
            nc.sync.dma_start(da[:, :, :], to_pf(da_d))

            nc.vector.memset(zrow[:, :], 0.0)
            zt = bass.AP(table.tensor, 0,
                         [[(TBL - 1) * COUT, 2], [1, COUT]])
            nc.sync.dma_start(zt, zrow[:, :])

            # ---- x0q = 0.1 * relu(x@W + b), computed per 128-row tile ----
            with tc.tile_pool(name="setup", bufs=1) as sp:
                wa = sp.tile([128, COUT], f32)
                wb = sp.tile([128, COUT], f32)
                wc = sp.tile([1, COUT], f32)
                ones = sp.tile([1, 128], f32)
                nc.sync.dma_start(wa[:, :], waug_d[0:128, :])
                nc.sync.dma_start(wb[:, :], waug_d[128:256, :])
                nc.sync.dma_start(wc[:, :], waug_d[256:257, :])
                nc.vector.memset(ones[:, :], 1.0)

                HT = 25 * 128  # first-half columns (25 tiles; second 24)
                for (c0, c1) in ((0, HT), (HT, SHP)):
                    xa = sp.tile([128, HT], f32, tag="xah", bufs=1)
                    xb = sp.tile([128, HT], f32, tag="xbh", bufs=1)
                    w = c1 - c0
                    nc.sync.dma_start(xa[:, :w], xt_d[0:128, c0:c1])
                    nc.sync.dma_start(xb[:, :w], xt_d[128:256, c0:c1])
                    for t in range(c0 // 128, c1 // 128):
                        po = psp.tile([128, COUT], f32, tag="po")
                        sl = slice(t * 128 - c0, (t + 1) * 128 - c0)
                        nc.tensor.matmul(po[:, :], xa[:, sl], wa[:, :],
                                         start=True, stop=False)
                        nc.tensor.matmul(po[:, :], xb[:, sl], wb[:, :],
                                         start=False, stop=False)
                        nc.tensor.matmul(po[:, :], ones[:, :], wc[:, :],
                                         start=False, stop=True)
                        nc.scalar.activation(
                            x0q[:, t, :], po[:, :],
                            mybir.ActivationFunctionType.Relu, scale=0.1)

                # cexp = dinv * x0q ; g0 = dinv * x0 = 10 * cexp
                di = gbp.tile([128, CH // 128, COUT], f32, tag="gb")
                nc.sync.dma_start(di[:, :TILES, :], to_pf(di_d))
                nc.vector.tensor_mul(cexp[:, :, :], di[:, :TILES, :],
                                     x0q[:, :, :])
                g0 = gbp.tile([128, CH // 128, COUT], f32, tag="gb")
                nc.vector.tensor_scalar_mul(g0[:, :TILES, :], cexp[:, :, :],
                                            10.0)
                nc.sync.dma_start(to_pf(ag_in), g0[:, :TILES, :])
            nc.gpsimd.collective_compute(
                "AllGather", mybir.AluOpType.bypass,
                replica_groups=[list(range(NC))],
                ins=[ag_in[:, :].opt()],
                outs=[table[1:1 + NC * SHP, :].opt()],
            )

            # ---- propagation steps ----
            for step in range(steps):
                nc.vector.memset(accL[:, :, :], 0.0)
                nc.vector.memset(accH[:, :, :], 0.0)
                for (idx_sb, chunks, acc, tbl_ap) in (
                    (idxh, chunksH, accH, table[HI_BASE:TBL, :]),
                    (idxl, chunksL, accL, table[0:LO_ROWS, :]),
                ):
                    for (a, ln, segs) in chunks:
                        gb = gbp.tile([128, CH // 128, COUT], f32, tag="gb")
                        nc.gpsimd.dma_gather(
                            out_ap=gb[:, :ln // 128, :],
                            in_ap=tbl_ap,
                            idxs_ap=idx_sb[:, a // 16:(a + ln) // 16],
                            num_idxs=ln,
                            num_idxs_reg=ln,
                            elem_size=COUT,
                            single_packet=False,
                        )
                        for (b0, b1, a0, a1) in segs:
                            nc.vector.tensor_add(
                                acc[:, a0:a1, :], acc[:, a0:a1, :],
                                gb[:, b0:b1, :])
                    if acc is accH:
                        # merge accH (rank_H order) into accL (rank_L order);
                        # issued before the L chunks so the bounce DMA +
                        # permutation gather overlap the L gather phase
                        nc.sync.dma_start(to_pf(hscr), accH[:, :, :])
                        permb = gbp.tile([128, TILES, COUT], f32,
                                         tag="pb", bufs=1)
                        nc.gpsimd.dma_gather(
                            out_ap=permb[:, :, :],
                            in_ap=hscr[:, :],
                            idxs_ap=idxp[:, :],
                            num_idxs=SHP,
                            num_idxs_reg=SHP,
                            elem_size=COUT,
                            single_packet=False,
                        )
                nc.vector.tensor_add(accL[:, :, :], accL[:, :, :],
                                     permb[:, :, :])

                gout = gbp.tile([128, CH // 128, COUT], f32, tag="gb")
                if step < steps - 1:
                    nc.vector.tensor_mul(gout[:, :TILES, :], accL[:, :, :],
                                         da[:, :, :])
                    nc.vector.tensor_add(gout[:, :TILES, :],
                                         gout[:, :TILES, :], cexp[:, :, :])
                    nc.sync.dma_start(to_pf(ag_in), gout[:, :TILES, :])
                    nc.gpsimd.collective_compute(
                        "AllGather", mybir.AluOpType.bypass,
                        replica_groups=[list(range(NC))],
                        ins=[ag_in[:, :].opt()],
                        outs=[table[1:1 + NC * SHP, :].opt()],
                    )
                else:
                    db = gbp.tile([128, CH // 128, COUT], f32, tag="gb")
                    nc.sync.dma_start(db[:, :TILES, :], to_pf(db_d))
                    nc.vector.tensor_mul(gout[:, :TILES, :], accL[:, :, :],
                                         db[:, :TILES, :])
                    nc.vector.tensor_add(gout[:, :TILES, :],
                                         gout[:, :TILES, :], x0q[:, :, :])
                    nc.sync.dma_start(to_pf(out_d), gout[:, :TILES, :])

    nc.compile()
    return nc


_GRAPH_CACHE = {}
LAST_RESULT = None


def kernel(x, edge_index, W, b):
    global LAST_RESULT
    from concourse.bass_utils import run_bass_kernel_spmd

    in_maps, sched, perms = _build_host(x, edge_index, W, b)
    NL, NH = sched
    if sched not in _GRAPH_CACHE:
        _GRAPH_CACHE[sched] = _build_graph(list(NL), list(NH))
    nc = _GRAPH_CACHE[sched]

    res = run_bass_kernel_spmd(nc, in_maps, core_ids=list(range(NC)))
    LAST_RESULT = res
    out = np.zeros((N, COUT), np.float32)
    for c in range(NC):
        out[c * SH + perms[c]] = res.results[c]["out"][:SH]
    return out


if __name__ == "__main__":
    x = np.load("/tmp/x.npy"); ei = np.load("/tmp/edge_index.npy")
    W = np.load("/tmp/W.npy"); b = np.load("/tmp/b.npy")
    actual = kernel(x, ei, W, b)
    expected = np.load("/tmp/expected.npy")
    rel = np.linalg.norm(actual - expected) / np.linalg.norm(expected)
    print("rel err:", rel)
